# revision 25
# baseline (speedup 1.0000x reference)
"""Multi-head attention (B=4, S=2048, M=1024, H=16, D=64) on 8 trn2 cores.

Sharding: core c = (b, g) with b = c // 2 (batch), g = c % 2 (head group of 8
heads).  Each core computes q/k/v projections for its 8 heads, causal
attention, and a partial output projection (contraction over its 512 feature
rows of Wo).  Host sums the two partials per batch and adds the bias.

The whole device datapath is bf16 (fp32 accumulation in PSUM): fp32(f32r)
matmuls stream the moving operand at half rate on trn2, so bf16 halves
TensorE time and DMA traffic.  exp() is batched two heads at a time into
[128, 1024] activations to amortize ScalarE's ~352-cycle fixed cost.

Device-side layouts:
  xq/xk/xv   [NJ*MK*128, 512] bf16  tile-packed activations: row block
                                    (j*MK+mc) = x.T[mc*128:+128, j*512:+512]
  wq/wk/wv   [1024(m), 512(dh)]     bf16, dh = 64*h_local + d (head-major)
  wo         [512(dh), 1024(n)]     bf16
  qT,kT      per (d,j): [128, 512]  bf16 = (X W)^T block
  v          [2048(s), 520] bf16    per 128-row tile: cols 65h..65h+63 = v_h,
                                    col 65h+64 = 1.0 (softmax denominator)
  sc2        [128(sk), 1024] PSUM   scores for head pair (hA|hB)
  pt2        exp((sc2 + mask)/8)    bf16 -- no max subtraction (|scores/8|<~2)
  pv_h       [65, 512] PSUM         = [v_h|1].T @ PT ; row 64 = sum_sk PT
  at         [128(dh), 512(sq)]     bf16 = pv/denom, head pair stacked
  out        [2048(s), 1024(n)] f32 partial = at.T @ wo (no bias)
"""

import os
import sys

for _p in ("/opt/trn_rl_repo", "/root/.axon_site/_ro/trn_rl_repo"):
    if os.path.isdir(_p) and _p not in sys.path:
        sys.path.append(_p)

import numpy as np

B, S, M, H, D = 4, 2048, 1024, 16, 64
G = 2               # head groups (cores per batch)
HPG = H // G        # heads per group = 8
DH = HPG * D        # feature rows per group = 512
NCORES = B * G
SB = 512            # sq block (matmul N)
CK = 128            # sk chunk (matmul M / partition)
NJ = S // SB        # 4 sq blocks
NC = S // CK        # 16 sk chunks
MK = M // 128       # 8 m chunks

_PROG_CACHE = {}


def _build_program(variant):
    """variant: 'causal' | 'allones' | 'general'"""
    import concourse.bass as bass
    import concourse.bacc as bacc
    import concourse.mybir as mybir
    from concourse import tile
    from contextlib import ExitStack

    f32 = mybir.dt.float32
    f32r = mybir.dt.float32r
    bf16 = mybir.dt.bfloat16
    nc = bacc.Bacc("TRN2", target_bir_lowering=False, debug=False, num_devices=NCORES)

    xq = nc.dram_tensor("xq", [NJ * MK * 128, SB], bf16, kind="ExternalInput").ap()
    xk = nc.dram_tensor("xk", [NJ * MK * 128, SB], bf16, kind="ExternalInput").ap()
    xv = nc.dram_tensor("xv", [NJ * MK * 128, SB], bf16, kind="ExternalInput").ap()
    wq = nc.dram_tensor("wq", [M, DH], bf16, kind="ExternalInput").ap()
    wk = nc.dram_tensor("wk", [M, DH], bf16, kind="ExternalInput").ap()
    wv = nc.dram_tensor("wv", [M, DH], bf16, kind="ExternalInput").ap()
    wo = nc.dram_tensor("wo", [DH, M], bf16, kind="ExternalInput").ap()
    tri2 = nc.dram_tensor("tri2", [128, 256], f32, kind="ExternalInput").ap()
    indh = nc.dram_tensor("indh", [2, 128], bf16, kind="ExternalInput").ap()
    if variant == "general":
        maskT = nc.dram_tensor("maskT", [S, S], f32, kind="ExternalInput").ap()
    out = nc.dram_tensor("out", [S, M], f32, kind="ExternalOutput").ap()

    with tile.TileContext(nc) as tc, ExitStack() as ctx:
        ep = ctx.enter_context
        ctx.enter_context(nc.allow_low_precision(reason="bf16 datapath"))
        dma = nc.sync.dma_start

        w_pool = ep(tc.tile_pool(name="w", bufs=1))       # unique names: resident
        wo_pool = ep(tc.tile_pool(name="wo", bufs=1))
        x_pool = ep(tc.tile_pool(name="x", bufs=26))
        qT_pool = ep(tc.tile_pool(name="qT", bufs=1))     # unique per (d, j)
        kT_pool = ep(tc.tile_pool(name="kT", bufs=1))
        v_pool = ep(tc.tile_pool(name="v", bufs=1))
        pt_pool = ep(tc.tile_pool(name="pt", bufs=6))
        at_pool = ep(tc.tile_pool(name="at", bufs=16))
        nrm_pool = ep(tc.tile_pool(name="nrm", bufs=3))
        out_pool = ep(tc.tile_pool(name="outp", bufs=6))
        misc_pool = ep(tc.tile_pool(name="misc", bufs=1))
        if variant == "general":
            mk_pool = ep(tc.tile_pool(name="mk", bufs=17))

        ps_sc = ep(tc.tile_pool(name="ps_sc", bufs=2, space="PSUM"))   # [128,1024] x2 = 4 banks
        ps_pv = ep(tc.tile_pool(name="ps_pv", bufs=2, space="PSUM"))   # 2 banks
        ps_mm = ep(tc.tile_pool(name="ps_mm", bufs=2, space="PSUM"))   # 2 banks

        # TensorE warm-up: ~40 dummy matmuls bridge the HAM clock-gate ramp
        # (~3.4us of sustained activity) while the first input DMAs land, so
        # the real projection matmuls start at 2.4 GHz instead of 1.2.
        wu_lhs = misc_pool.tile([128, 128], bf16, name="wu_lhs")
        nc.gpsimd.memset(wu_lhs[:], 0.0)
        wu_rhs = misc_pool.tile([128, SB], bf16, name="wu_rhs")
        nc.gpsimd.memset(wu_rhs[:], 0.0)
        wu_ps = ps_sc.tile([128, SB], f32, name="wu_ps", tag="sc")
        for _ in range(40):
            nc.tensor.matmul(wu_ps[:], wu_lhs[:], wu_rhs[:], start=True, stop=True)

        # constants
        tri2_sb = misc_pool.tile([128, 256], f32, name="tri2_sb")
        dma(tri2_sb[:], tri2[:])
        ind2_sb = misc_pool.tile([2, 128], bf16, name="ind2_sb")
        dma(ind2_sb[:], indh[:])
        # resident weights, interleaved with the j=0 activation prefetch so
        # the first projection matmuls aren't stuck behind 4 MB of weights
        w_sb = {}
        x_prefetch = {}
        for w_dram, x_dram, kind in ((wq, xq, "q"), (wk, xk, "k"), (wv, xv, "v")):
            ch = []
            for mc in range(MK):
                wt = w_pool.tile([128, DH], bf16, name=f"w_{kind}_{mc}")
                nc.gpsimd.dma_start(wt[:], w_dram[mc * 128:(mc + 1) * 128, :])
                ch.append(wt)
            w_sb[kind] = ch
            xch = []
            for mc in range(MK):
                xt = x_pool.tile([128, SB], bf16, name=f"x_{kind}0_{mc}", tag="x")
                dma(xt[:], x_dram[mc * 128:mc * 128 + 128, :])
                xch.append(xt)
            x_prefetch[kind] = xch
        wo_sb = []
        for d in range(4):
            wt = wo_pool.tile([128, M], bf16, name=f"wo{d}")
            nc.gpsimd.dma_start(wt[:], wo[d * 128:(d + 1) * 128, :])
            wo_sb.append(wt)

        # persistent activation tiles
        qT_sb = [[qT_pool.tile([128, SB], bf16, name=f"qT{d}_{j}")
                  for j in range(NJ)] for d in range(4)]
        kT_sb = [[kT_pool.tile([128, SB], bf16, name=f"kT{d}_{j}")
                  for j in range(NJ)] for d in range(4)]
        v_sb = [v_pool.tile([128, HPG * 65], bf16, name=f"v{t}") for t in range(NC)]

        for t in range(NC):
            v3 = v_sb[t].rearrange("p (h c) -> p h c", h=HPG, c=65)
            nc.gpsimd.memset(v3[:, :, 64:65], 1.0)

        # ---- phase 1: projections, per s-block ----
        x_cache = {}
        def emit_proj(j, kinds="qkv", dsel=None):
            for x_dram, kind in ((xq, "q"), (xk, "k"), (xv, "v")):
                if kind not in kinds:
                    continue
                if (kind, j) in x_cache:
                    x_ch = x_cache[(kind, j)]
                elif j == 0:
                    x_ch = x_cache[(kind, j)] = x_prefetch.pop(kind)
                else:
                    x_ch = []
                    for mc in range(MK):
                        xt = x_pool.tile([128, SB], bf16, name=f"x_{kind}{j}_{mc}", tag="x")
                        r0 = (j * MK + mc) * 128
                        dma(xt[:], x_dram[r0:r0 + 128, :])
                        x_ch.append(xt)
                    x_cache[(kind, j)] = x_ch
                w_ch = w_sb[kind]
                if kind in ("q", "k"):
                    dst = qT_sb if kind == "q" else kT_sb
                    for d in (range(4) if dsel is None else dsel):
                        ps = ps_mm.tile([128, SB], f32, name=f"ps_{kind}{j}_{d}", tag="mm")
                        for mc in range(MK):
                            nc.tensor.matmul(
                                ps[:], w_ch[mc][:, d * 128:(d + 1) * 128], x_ch[mc][:],
                                start=(mc == 0), stop=(mc == MK - 1))
                        nc.vector.tensor_copy(dst[d][j][:], ps[:])
                else:
                    for st in range(4):
                        t = 4 * j + st
                        ps = ps_mm.tile([128, DH], f32, name=f"ps_v{t}", tag="mm")
                        for mc in range(MK):
                            nc.tensor.matmul(
                                ps[:], x_ch[mc][:, st * 128:(st + 1) * 128], w_ch[mc][:],
                                start=(mc == 0), stop=(mc == MK - 1))
                        v3 = v_sb[t].rearrange("p (h c) -> p h c", h=HPG, c=65)
                        p3 = ps.rearrange("p (h c) -> p h c", h=HPG, c=64)
                        nc.vector.tensor_copy(v3[:, :, 0:64], p3[:])

        # ---- phase 2: attention + output projection, per sq block ----
        def emit_attn(j, pre_hp=None):
            nchunks = 4 * (j + 1) if variant == "causal" else NC
            if variant == "general":
                mk_tiles = []
                for c in range(nchunks):
                    mk = mk_pool.tile([128, SB], f32, name=f"mk{j}_{c}", tag="mk")
                    nc.gpsimd.dma_start(
                        mk[:], maskT[c * CK:(c + 1) * CK, j * SB:(j + 1) * SB])
                    mk_tiles.append(mk)
            at_tiles = []
            for hp in range(HPG // 2):
                if pre_hp is not None:
                    pre_hp(hp)
                hA, hB = 2 * hp, 2 * hp + 1
                at = at_pool.tile([128, SB], bf16, name=f"at{j}_{hp}", tag="at")
                at_tiles.append(at)
                rb2 = ps_pv.tile([128, SB], f32, name=f"rb{j}_{hp}", tag="pv")
                pvA = ps_pv.tile([65, SB], f32, name=f"pv{j}_{hA}", tag="pv")
                pvB = ps_pv.tile([65, SB], f32, name=f"pv{j}_{hB}", tag="pv")
                for c in range(nchunks):
                    o = 0
                    if variant == "causal" and c >= 4 * j:
                        o = 128 * (c - 4 * j)
                    # narrow mostly-masked diagonal chunks; exp then reads
                    # only the written region (split calls below)
                    lo = o if o >= 256 else 0
                    sc2 = ps_sc.tile([128, 2 * SB], f32, name=f"sc{j}_{hp}_{c}", tag="sc")
                    for hi, h in enumerate((hA, hB)):
                        dtile, drow = h // 2, 64 * (h % 2)
                        nc.tensor.matmul(
                            sc2[:, hi * SB + lo:(hi + 1) * SB],
                            kT_sb[dtile][c // 4][drow:drow + 64,
                                                 (c % 4) * CK:(c % 4 + 1) * CK],
                            qT_sb[dtile][j][drow:drow + 64, lo:],
                            start=True, stop=True)
                    if variant == "causal" and c >= 4 * j:
                        sc2v = sc2.rearrange("p (t q) -> p t q", t=2)[:, :, o:o + 128]
                        tri2v = tri2_sb.rearrange("p (t q) -> p t q", t=2)
                        nc.vector.tensor_add(sc2v, sc2v, tri2v)
                    elif variant == "general":
                        for hi in range(2):
                            nc.vector.tensor_add(
                                sc2[:, hi * SB:(hi + 1) * SB],
                                sc2[:, hi * SB:(hi + 1) * SB], mk_tiles[c][:])
                    pt2 = pt_pool.tile([128, 2 * SB], bf16, name=f"pt{j}_{hp}_{c}", tag="pt")
                    if lo > 0:
                        for hi in range(2):
                            nc.scalar.activation(
                                pt2[:, hi * SB + lo:(hi + 1) * SB],
                                sc2[:, hi * SB + lo:(hi + 1) * SB],
                                mybir.ActivationFunctionType.Exp, scale=0.125)
                    else:
                        nc.scalar.activation(
                            pt2[:], sc2[:],
                            mybir.ActivationFunctionType.Exp, scale=0.125)
                    nc.tensor.matmul(
                        pvA[:, o:SB], v_sb[c][:, 65 * hA:65 * hA + 65],
                        pt2[:, o:SB],
                        start=(c == 0), stop=(c == nchunks - 1))
                    nc.tensor.matmul(
                        pvB[:, o:SB], v_sb[c][:, 65 * hB:65 * hB + 65],
                        pt2[:, SB + o:2 * SB],
                        start=(c == 0), stop=(c == nchunks - 1))
                # denominator chain first (it gates the rb2 matmul), the
                # bulk at-copies second
                dn2b = nrm_pool.tile([2, SB], bf16, name=f"dn2b{j}_{hp}", tag="dn2b")
                for hi, (h, pv) in enumerate(((hA, pvA), (hB, pvB))):
                    dnt = nrm_pool.tile([1, SB], f32, name=f"dnt{j}_{h}", tag="dnt")
                    nc.vector.tensor_copy(dnt[:], pv[64:65, :])
                    dnr = nrm_pool.tile([1, SB], f32, name=f"dnr{j}_{h}", tag="dnr")
                    nc.vector.reciprocal_approx_fast(dnr[:], dnt[:])
                    if hi == 0:
                        nc.vector.tensor_copy(dn2b[0:1, :], dnr[:])
                    else:
                        dnrb = nrm_pool.tile([1, SB], bf16, name=f"dnrb{j}_{h}", tag="dnrb")
                        nc.vector.tensor_copy(dnrb[:], dnr[:])
                        nc.gpsimd.dma_start(dn2b[1:2, :], dnrb[:])
                # rank-2 broadcast: rows [0:64)=1/denomA, [64:128)=1/denomB
                nc.tensor.matmul(rb2[:], ind2_sb[:], dn2b[:], start=True, stop=True)
                for h, pv in ((hA, pvA), (hB, pvB)):
                    drow = 64 * (h % 2)
                    nc.vector.tensor_copy(at[drow:drow + 64, :], pv[0:64, :])
                nc.vector.tensor_mul(at[:], at[:], rb2[:])
            return at_tiles

        def emit_outproj(j, at_tiles):
            for ss in range(4):
                for nh in range(2):
                    ps = ps_mm.tile([128, SB], f32, name=f"po{j}_{ss}_{nh}", tag="mm")
                    for d in range(4):
                        nc.tensor.matmul(
                            ps[:],
                            at_tiles[d][:, ss * 128:(ss + 1) * 128],
                            wo_sb[d][:, nh * SB:(nh + 1) * SB],
                            start=(d == 0), stop=(d == 3))
                    ot = out_pool.tile([128, SB], f32, name=f"ot{j}_{ss}_{nh}", tag="ot")
                    if j == NJ - 1:
                        nc.vector.tensor_copy(ot[:], ps[:])
                    else:
                        nc.scalar.activation(
                            ot[:], ps[:], mybir.ActivationFunctionType.Copy)
                    r0 = j * SB + ss * 128
                    nc.gpsimd.dma_start(
                        out[r0:r0 + 128, nh * SB:(nh + 1) * SB], ot[:])

        # Interleaved emission: the tile scheduler dispatches by readiness,
        # but emission order sets priority -- emit attn(j) before proj(j+1)
        # so ScalarE fills early, and let proj/outproj MMs fill TensorE's
        # exp-wait gaps (also keeps the PE HAM clock-gate warm).  attn(0) is
        # emitted before the v projection of block 0: its score/exp work only
        # needs q/k, so ScalarE starts ~20us earlier (pv matmuls wait on v
        # via dataflow).
        emit_proj(0, kinds="qk", dsel=[0])
        emit_proj(0, kinds="v")

        def pre_hp0(hp):
            if hp >= 1:
                emit_proj(0, kinds="qk", dsel=[hp])

        # outproj(j) emission is deferred: attn(3) is ACT(exp)-paced, so the
        # out-projection matmuls are saved up as TensorE gap-filler for the
        # tail (keeps the PE dense and the HAM clock-gate warm there).
        ats = []
        for j in range(NJ):
            with tc.high_priority(offset=200000):
                ats.append(emit_attn(j, pre_hp=pre_hp0 if j == 0 else None))
            if j == 2:
                emit_outproj(0, ats[0])
            if j + 1 < NJ:
                emit_proj(j + 1)
        for j in range(1, NJ):
            emit_outproj(j, ats[j])

    nc.compile()
    return nc


def _get_program(variant):
    if variant not in _PROG_CACHE:
        _PROG_CACHE[variant] = _build_program(variant)
    return _PROG_CACHE[variant]


def _pack_x(x, bf16):
    """x [S, M] fp32 -> [NJ*MK*128, SB] bf16 tile-packed (row block j*MK+mc
    holds x.T[mc*128:+128, j*SB:+SB], i.e. x[j*SB:+SB, mc*128:+128].T)."""
    x = np.asarray(x, np.float32).astype(bf16)
    return np.ascontiguousarray(
        x.reshape(NJ, SB, MK, 128).transpose(0, 2, 3, 1).reshape(NJ * MK * 128, SB))


def _host_prep(queries, keys, values, masks, Wq, Wk, Wv):
    """Build the 8 per-core input maps."""
    import ml_dtypes
    bf16 = ml_dtypes.bfloat16

    tril = np.tril(np.ones((S, S), dtype=bool))
    if all(np.array_equal(masks[b], tril) for b in range(B)):
        variant = "causal"
    elif masks.all():
        variant = "allones"
    else:
        variant = "general"

    sq = np.arange(128)
    tri_np = np.where(sq[None, :] >= sq[:, None], 0.0, -1.0e6).astype(np.float32)
    tri2_np = np.ascontiguousarray(np.concatenate([tri_np, tri_np], axis=1))
    indh_np = np.zeros((2, 128), np.float32)
    indh_np[0, 0:64] = 1.0
    indh_np[1, 64:128] = 1.0
    indh_np = indh_np.astype(bf16)

    # [H, M, D] -> [M, H*D] head-major per group
    def wcat(w, g):
        return np.ascontiguousarray(
            w[g * HPG:(g + 1) * HPG].transpose(1, 0, 2).reshape(M, DH)).astype(bf16)

    xqs = [_pack_x(queries[b], bf16) for b in range(B)]
    xks = [_pack_x(keys[b], bf16) for b in range(B)]
    xvs = [_pack_x(values[b], bf16) for b in range(B)]
    wqs = [wcat(Wq, g) for g in range(G)]
    wks = [wcat(Wk, g) for g in range(G)]
    wvs = [wcat(Wv, g) for g in range(G)]
    if variant == "general":
        maskTs = [np.where(masks[b].T, 0.0, -1.0e6).astype(np.float32)
                  for b in range(B)]

    in_maps = []
    for c in range(NCORES):
        b, g = c // G, c % G
        m = {
            "xq": xqs[b], "xk": xks[b], "xv": xvs[b],
            "wq": wqs[g], "wk": wks[g], "wv": wvs[g],
            "tri2": tri2_np,
            "indh": indh_np,
        }
        if variant == "general":
            m["maskT"] = maskTs[b]
        in_maps.append(m)
    return variant, in_maps


def run(queries, keys, values, masks, Wq, Wk, Wv, Wo, bo, trace=False):
    from concourse import bass_utils
    import ml_dtypes
    bf16 = ml_dtypes.bfloat16

    queries = np.asarray(queries, np.float32)
    keys = np.asarray(keys, np.float32)
    values = np.asarray(values, np.float32)
    masks = np.asarray(masks, bool)
    Wq = np.asarray(Wq, np.float32)
    Wk = np.asarray(Wk, np.float32)
    Wv = np.asarray(Wv, np.float32)
    Wo = np.asarray(Wo, np.float32)
    bo = np.asarray(bo, np.float32)

    variant, in_maps = _host_prep(queries, keys, values, masks, Wq, Wk, Wv)
    wos = [np.ascontiguousarray(Wo[g * DH:(g + 1) * DH, :]).astype(bf16)
           for g in range(G)]
    for c in range(NCORES):
        in_maps[c]["wo"] = wos[c % G]

    nc = _get_program(variant)
    res = bass_utils.run_bass_kernel_spmd(
        nc, in_maps, list(range(NCORES)), trace=trace)

    out = np.empty((B, S, M), np.float32)
    for b in range(B):
        out[b] = res.results[G * b]["out"] + res.results[G * b + 1]["out"] + bo
    return out, res


def kernel(queries, keys, values, masks, Wq, Wk, Wv, Wo, bo):
    out, _ = run(queries, keys, values, masks, Wq, Wk, Wv, Wo, bo, trace=False)
    return out


# revision 27
# speedup vs baseline: 1.0493x; 1.0493x over previous
"""Multi-head attention (B=4, S=2048, M=1024, H=16, D=64) on 8 trn2 cores.

Sharding: core c = (b, g) with b = c // 2 (batch), g = c % 2 (head group of 8
heads).  Each core computes q/k/v projections for its 8 heads, causal
attention, and a partial output projection (contraction over its 512 feature
rows of Wo).  Host sums the two partials per batch and adds the bias.

The whole device datapath is bf16 (fp32 accumulation in PSUM): fp32(f32r)
matmuls stream the moving operand at half rate on trn2, so bf16 halves
TensorE time and DMA traffic.  exp() is batched two heads at a time into
[128, 1024] activations to amortize ScalarE's ~352-cycle fixed cost.

Device-side layouts:
  xq/xk/xv   [NJ*MK*128, 512] bf16  tile-packed activations: row block
                                    (j*MK+mc) = x.T[mc*128:+128, j*512:+512]
  wq/wk/wv   [1024(m), 512(dh)]     bf16, dh = 64*h_local + d (head-major)
  wo         [512(dh), 1024(n)]     bf16
  qT,kT      per (d,j): [128, 512]  bf16 = (X W)^T block
  v          [2048(s), 520] bf16    per 128-row tile: cols 65h..65h+63 = v_h,
                                    col 65h+64 = 1.0 (softmax denominator)
  sc2        [128(sk), 1024] PSUM   scores for head pair (hA|hB)
  pt2        exp((sc2 + mask)/8)    bf16 -- no max subtraction (|scores/8|<~2)
  pv_h       [65, 512] PSUM         = [v_h|1].T @ PT ; row 64 = sum_sk PT
  at         [128(dh), 512(sq)]     bf16 = pv/denom, head pair stacked
  out        [2048(s), 1024(n)] f32 partial = at.T @ wo (no bias)
"""

import os
import sys

for _p in ("/opt/trn_rl_repo", "/root/.axon_site/_ro/trn_rl_repo"):
    if os.path.isdir(_p) and _p not in sys.path:
        sys.path.append(_p)

import numpy as np

B, S, M, H, D = 4, 2048, 1024, 16, 64
G = 2               # head groups (cores per batch)
HPG = H // G        # heads per group = 8
DH = HPG * D        # feature rows per group = 512
NCORES = B * G
SB = 512            # sq block (matmul N)
CK = 128            # sk chunk (matmul M / partition)
NJ = S // SB        # 4 sq blocks
NC = S // CK        # 16 sk chunks
MK = M // 128       # 8 m chunks

_PROG_CACHE = {}


def _build_program(variant):
    """variant: 'causal' | 'allones' | 'general'"""
    import concourse.bass as bass
    import concourse.bacc as bacc
    import concourse.mybir as mybir
    from concourse import tile
    from contextlib import ExitStack

    f32 = mybir.dt.float32
    f32r = mybir.dt.float32r
    bf16 = mybir.dt.bfloat16
    nc = bacc.Bacc("TRN2", target_bir_lowering=False, debug=False, num_devices=NCORES)

    xq = nc.dram_tensor("xq", [NJ * MK * 128, SB], bf16, kind="ExternalInput").ap()
    xk = nc.dram_tensor("xk", [NJ * MK * 128, SB], bf16, kind="ExternalInput").ap()
    xv = nc.dram_tensor("xv", [NJ * MK * 128, SB], bf16, kind="ExternalInput").ap()
    wq = nc.dram_tensor("wq", [M, DH], bf16, kind="ExternalInput").ap()
    wk = nc.dram_tensor("wk", [M, DH], bf16, kind="ExternalInput").ap()
    wv = nc.dram_tensor("wv", [M, DH], bf16, kind="ExternalInput").ap()
    wo = nc.dram_tensor("wo", [DH, M], bf16, kind="ExternalInput").ap()
    tri2 = nc.dram_tensor("tri2", [128, 256], f32, kind="ExternalInput").ap()
    indh = nc.dram_tensor("indh", [2, 128], bf16, kind="ExternalInput").ap()
    if variant == "general":
        maskT = nc.dram_tensor("maskT", [S, S], f32, kind="ExternalInput").ap()
    out = nc.dram_tensor("out", [S, M], f32, kind="ExternalOutput").ap()

    with tile.TileContext(nc) as tc, ExitStack() as ctx:
        ep = ctx.enter_context
        ctx.enter_context(nc.allow_low_precision(reason="bf16 datapath"))
        dma = nc.sync.dma_start

        w_pool = ep(tc.tile_pool(name="w", bufs=1))       # unique names: resident
        wo_pool = ep(tc.tile_pool(name="wo", bufs=1))
        x_pool = ep(tc.tile_pool(name="x", bufs=34))
        qT_pool = ep(tc.tile_pool(name="qT", bufs=1))     # unique per (d, j)
        kT_pool = ep(tc.tile_pool(name="kT", bufs=1))
        v_pool = ep(tc.tile_pool(name="v", bufs=1))
        pt_pool = ep(tc.tile_pool(name="pt", bufs=8))
        at_pool = ep(tc.tile_pool(name="at", bufs=16))
        nrm_pool = ep(tc.tile_pool(name="nrm", bufs=4))
        out_pool = ep(tc.tile_pool(name="outp", bufs=6))
        misc_pool = ep(tc.tile_pool(name="misc", bufs=1))
        if variant == "general":
            mk_pool = ep(tc.tile_pool(name="mk", bufs=17))

        ps_sc = ep(tc.tile_pool(name="ps_sc", bufs=2, space="PSUM"))   # [128,1024] x2 = 4 banks
        ps_pv = ep(tc.tile_pool(name="ps_pv", bufs=2, space="PSUM"))   # 2 banks
        ps_mm = ep(tc.tile_pool(name="ps_mm", bufs=2, space="PSUM"))   # 2 banks

        # TensorE warm-up: ~40 dummy matmuls bridge the HAM clock-gate ramp
        # (~3.4us of sustained activity) while the first input DMAs land, so
        # the real projection matmuls start at 2.4 GHz instead of 1.2.
        wu_lhs = misc_pool.tile([128, 128], bf16, name="wu_lhs")
        nc.gpsimd.memset(wu_lhs[:], 0.0)
        wu_rhs = misc_pool.tile([128, SB], bf16, name="wu_rhs")
        nc.gpsimd.memset(wu_rhs[:], 0.0)
        wu_ps = ps_sc.tile([128, SB], f32, name="wu_ps", tag="sc")
        for _ in range(40):
            nc.tensor.matmul(wu_ps[:], wu_lhs[:], wu_rhs[:], start=True, stop=True)

        # constants
        tri2_sb = misc_pool.tile([128, 256], f32, name="tri2_sb")
        dma(tri2_sb[:], tri2[:])
        indA_sb = misc_pool.tile([1, 128], bf16, name="indA_sb")
        dma(indA_sb[:], indh[0:1, :])
        indB_sb = misc_pool.tile([1, 128], bf16, name="indB_sb")
        dma(indB_sb[:], indh[1:2, :])
        # resident weights, interleaved with the j=0 activation prefetch so
        # the first projection matmuls aren't stuck behind 4 MB of weights
        w_sb = {}
        x_prefetch = {}
        for w_dram, x_dram, kind in ((wq, xq, "q"), (wk, xk, "k"), (wv, xv, "v")):
            ch = []
            for mc in range(MK):
                wt = w_pool.tile([128, DH], bf16, name=f"w_{kind}_{mc}")
                nc.gpsimd.dma_start(wt[:], w_dram[mc * 128:(mc + 1) * 128, :])
                ch.append(wt)
            w_sb[kind] = ch
            xch = []
            for mc in range(MK):
                xt = x_pool.tile([128, SB], bf16, name=f"x_{kind}0_{mc}", tag="x")
                dma(xt[:], x_dram[mc * 128:mc * 128 + 128, :])
                xch.append(xt)
            x_prefetch[kind] = xch
        wo_sb = []
        for d in range(4):
            wt = wo_pool.tile([128, M], bf16, name=f"wo{d}")
            nc.gpsimd.dma_start(wt[:], wo[d * 128:(d + 1) * 128, :])
            wo_sb.append(wt)

        # persistent activation tiles
        qT_sb = [[qT_pool.tile([128, SB], bf16, name=f"qT{d}_{j}")
                  for j in range(NJ)] for d in range(4)]
        kT_sb = [[kT_pool.tile([128, SB], bf16, name=f"kT{d}_{j}")
                  for j in range(NJ)] for d in range(4)]
        v_sb = [v_pool.tile([128, HPG * 65], bf16, name=f"v{t}") for t in range(NC)]

        for t in range(NC):
            v3 = v_sb[t].rearrange("p (h c) -> p h c", h=HPG, c=65)
            nc.gpsimd.memset(v3[:, :, 64:65], 1.0)

        # ---- phase 1: projections, per s-block ----
        x_cache = {}
        def emit_proj(j, kinds="qkv", dsel=None):
            for x_dram, kind in ((xq, "q"), (xk, "k"), (xv, "v")):
                if kind not in kinds:
                    continue
                if (kind, j) in x_cache:
                    x_ch = x_cache[(kind, j)]
                elif j == 0:
                    x_ch = x_cache[(kind, j)] = x_prefetch.pop(kind)
                else:
                    x_ch = []
                    for mc in range(MK):
                        xt = x_pool.tile([128, SB], bf16, name=f"x_{kind}{j}_{mc}", tag="x")
                        r0 = (j * MK + mc) * 128
                        dma(xt[:], x_dram[r0:r0 + 128, :])
                        x_ch.append(xt)
                    x_cache[(kind, j)] = x_ch
                w_ch = w_sb[kind]
                if kind in ("q", "k"):
                    dst = qT_sb if kind == "q" else kT_sb
                    for d in (range(4) if dsel is None else dsel):
                        ps = ps_mm.tile([128, SB], f32, name=f"ps_{kind}{j}_{d}", tag="mm")
                        for mc in range(MK):
                            nc.tensor.matmul(
                                ps[:], w_ch[mc][:, d * 128:(d + 1) * 128], x_ch[mc][:],
                                start=(mc == 0), stop=(mc == MK - 1))
                        nc.vector.tensor_copy(dst[d][j][:], ps[:])
                else:
                    for st in range(4):
                        t = 4 * j + st
                        ps = ps_mm.tile([128, DH], f32, name=f"ps_v{t}", tag="mm")
                        for mc in range(MK):
                            nc.tensor.matmul(
                                ps[:], x_ch[mc][:, st * 128:(st + 1) * 128], w_ch[mc][:],
                                start=(mc == 0), stop=(mc == MK - 1))
                        v3 = v_sb[t].rearrange("p (h c) -> p h c", h=HPG, c=65)
                        p3 = ps.rearrange("p (h c) -> p h c", h=HPG, c=64)
                        nc.vector.tensor_copy(v3[:, :, 0:64], p3[:])

        # ---- phase 2: attention + output projection, per sq block ----
        def emit_attn(j, pre_hp=None):
            nchunks = 4 * (j + 1) if variant == "causal" else NC
            if variant == "general":
                mk_tiles = []
                for c in range(nchunks):
                    mk = mk_pool.tile([128, SB], f32, name=f"mk{j}_{c}", tag="mk")
                    nc.gpsimd.dma_start(
                        mk[:], maskT[c * CK:(c + 1) * CK, j * SB:(j + 1) * SB])
                    mk_tiles.append(mk)
            at_tiles = []
            for hp in range(HPG // 2):
                if pre_hp is not None:
                    pre_hp(hp)
                hA, hB = 2 * hp, 2 * hp + 1
                at = at_pool.tile([128, SB], bf16, name=f"at{j}_{hp}", tag="at")
                at_tiles.append(at)
                rb2 = ps_pv.tile([128, SB], f32, name=f"rb{j}_{hp}", tag="pv")
                pvA = ps_pv.tile([65, SB], f32, name=f"pv{j}_{hA}", tag="pv")
                pvB = ps_pv.tile([65, SB], f32, name=f"pv{j}_{hB}", tag="pv")
                for c in range(nchunks):
                    o = 0
                    if variant == "causal" and c >= 4 * j:
                        o = 128 * (c - 4 * j)
                    # narrow mostly-masked diagonal chunks; exp then reads
                    # only the written region (split calls below)
                    lo = o if o >= 256 else 0
                    sc2 = ps_sc.tile([128, 2 * SB], f32, name=f"sc{j}_{hp}_{c}", tag="sc")
                    for hi, h in enumerate((hA, hB)):
                        dtile, drow = h // 2, 64 * (h % 2)
                        nc.tensor.matmul(
                            sc2[:, hi * SB + lo:(hi + 1) * SB],
                            kT_sb[dtile][c // 4][drow:drow + 64,
                                                 (c % 4) * CK:(c % 4 + 1) * CK],
                            qT_sb[dtile][j][drow:drow + 64, lo:],
                            start=True, stop=True)
                    if variant == "causal" and c >= 4 * j:
                        sc2v = sc2.rearrange("p (t q) -> p t q", t=2)[:, :, o:o + 128]
                        tri2v = tri2_sb.rearrange("p (t q) -> p t q", t=2)
                        nc.vector.tensor_add(sc2v, sc2v, tri2v)
                    elif variant == "general":
                        for hi in range(2):
                            nc.vector.tensor_add(
                                sc2[:, hi * SB:(hi + 1) * SB],
                                sc2[:, hi * SB:(hi + 1) * SB], mk_tiles[c][:])
                    pt2 = pt_pool.tile([128, 2 * SB], bf16, name=f"pt{j}_{hp}_{c}", tag="pt")
                    if lo > 0:
                        for hi in range(2):
                            nc.scalar.activation(
                                pt2[:, hi * SB + lo:(hi + 1) * SB],
                                sc2[:, hi * SB + lo:(hi + 1) * SB],
                                mybir.ActivationFunctionType.Exp, scale=0.125)
                    else:
                        nc.scalar.activation(
                            pt2[:], sc2[:],
                            mybir.ActivationFunctionType.Exp, scale=0.125)
                    nc.tensor.matmul(
                        pvA[:, o:SB], v_sb[c][:, 65 * hA:65 * hA + 65],
                        pt2[:, o:SB],
                        start=(c == 0), stop=(c == nchunks - 1))
                    nc.tensor.matmul(
                        pvB[:, o:SB], v_sb[c][:, 65 * hB:65 * hB + 65],
                        pt2[:, SB + o:2 * SB],
                        start=(c == 0), stop=(c == nchunks - 1))
                # denominator chain first (it gates the rb2 matmul), the
                # bulk at-copies second
                for hi, (h, pv, ind) in enumerate(
                        ((hA, pvA, indA_sb), (hB, pvB, indB_sb))):
                    dnt = nrm_pool.tile([1, SB], f32, name=f"dnt{j}_{h}", tag="dnt")
                    nc.vector.tensor_copy(dnt[:], pv[64:65, :])
                    dnr = nrm_pool.tile([1, SB], f32, name=f"dnr{j}_{h}", tag="dnr")
                    nc.vector.reciprocal_approx_fast(dnr[:], dnt[:])
                    dnrb = nrm_pool.tile([1, SB], bf16, name=f"dnrb{j}_{h}", tag="dnrb")
                    nc.vector.tensor_copy(dnrb[:], dnr[:])
                    # rank-1 broadcast: rows [64*hi, 64*hi+64) of rb2 = 1/denom
                    nc.tensor.matmul(
                        rb2[:], ind[:], dnrb[:],
                        start=(hi == 0), stop=(hi == 1))
                for h, pv in ((hA, pvA), (hB, pvB)):
                    drow = 64 * (h % 2)
                    nc.vector.tensor_copy(at[drow:drow + 64, :], pv[0:64, :])
                nc.vector.tensor_mul(at[:], at[:], rb2[:])
            return at_tiles

        def emit_outproj(j, at_tiles):
            for ss in range(4):
                for nh in range(2):
                    ps = ps_mm.tile([128, SB], f32, name=f"po{j}_{ss}_{nh}", tag="mm")
                    for d in range(4):
                        nc.tensor.matmul(
                            ps[:],
                            at_tiles[d][:, ss * 128:(ss + 1) * 128],
                            wo_sb[d][:, nh * SB:(nh + 1) * SB],
                            start=(d == 0), stop=(d == 3))
                    ot = out_pool.tile([128, SB], f32, name=f"ot{j}_{ss}_{nh}", tag="ot")
                    if j == NJ - 1:
                        nc.vector.tensor_copy(ot[:], ps[:])
                    else:
                        nc.scalar.activation(
                            ot[:], ps[:], mybir.ActivationFunctionType.Copy)
                    r0 = j * SB + ss * 128
                    nc.gpsimd.dma_start(
                        out[r0:r0 + 128, nh * SB:(nh + 1) * SB], ot[:])

        # Interleaved emission: the tile scheduler dispatches by readiness,
        # but emission order sets priority -- emit attn(j) before proj(j+1)
        # so ScalarE fills early, and let proj/outproj MMs fill TensorE's
        # exp-wait gaps (also keeps the PE HAM clock-gate warm).  attn(0) is
        # emitted before the v projection of block 0: its score/exp work only
        # needs q/k, so ScalarE starts ~20us earlier (pv matmuls wait on v
        # via dataflow).
        emit_proj(0, kinds="qk", dsel=[0])
        emit_proj(0, kinds="v")

        def pre_hp0(hp):
            if hp >= 1:
                emit_proj(0, kinds="qk", dsel=[hp])

        # outproj(j) emission is deferred: attn(3) is ACT(exp)-paced, so the
        # out-projection matmuls are saved up as TensorE gap-filler for the
        # tail (keeps the PE dense and the HAM clock-gate warm there).
        ats = []
        for j in range(NJ):
            with tc.high_priority(offset=200000):
                ats.append(emit_attn(j, pre_hp=pre_hp0 if j == 0 else None))
            if j == 2:
                emit_outproj(0, ats[0])
            if j + 1 < NJ:
                emit_proj(j + 1)
        for j in range(1, NJ):
            emit_outproj(j, ats[j])

    nc.compile()
    return nc


def _get_program(variant):
    if variant not in _PROG_CACHE:
        _PROG_CACHE[variant] = _build_program(variant)
    return _PROG_CACHE[variant]


def _pack_x(x, bf16):
    """x [S, M] fp32 -> [NJ*MK*128, SB] bf16 tile-packed (row block j*MK+mc
    holds x.T[mc*128:+128, j*SB:+SB], i.e. x[j*SB:+SB, mc*128:+128].T)."""
    x = np.asarray(x, np.float32).astype(bf16)
    return np.ascontiguousarray(
        x.reshape(NJ, SB, MK, 128).transpose(0, 2, 3, 1).reshape(NJ * MK * 128, SB))


def _host_prep(queries, keys, values, masks, Wq, Wk, Wv):
    """Build the 8 per-core input maps."""
    import ml_dtypes
    bf16 = ml_dtypes.bfloat16

    tril = np.tril(np.ones((S, S), dtype=bool))
    if all(np.array_equal(masks[b], tril) for b in range(B)):
        variant = "causal"
    elif masks.all():
        variant = "allones"
    else:
        variant = "general"

    sq = np.arange(128)
    tri_np = np.where(sq[None, :] >= sq[:, None], 0.0, -1.0e6).astype(np.float32)
    tri2_np = np.ascontiguousarray(np.concatenate([tri_np, tri_np], axis=1))
    indh_np = np.zeros((2, 128), np.float32)
    indh_np[0, 0:64] = 1.0
    indh_np[1, 64:128] = 1.0
    indh_np = indh_np.astype(bf16)

    # [H, M, D] -> [M, H*D] head-major per group
    def wcat(w, g):
        return np.ascontiguousarray(
            w[g * HPG:(g + 1) * HPG].transpose(1, 0, 2).reshape(M, DH)).astype(bf16)

    xqs = [_pack_x(queries[b], bf16) for b in range(B)]
    xks = [_pack_x(keys[b], bf16) for b in range(B)]
    xvs = [_pack_x(values[b], bf16) for b in range(B)]
    wqs = [wcat(Wq, g) for g in range(G)]
    wks = [wcat(Wk, g) for g in range(G)]
    wvs = [wcat(Wv, g) for g in range(G)]
    if variant == "general":
        maskTs = [np.where(masks[b].T, 0.0, -1.0e6).astype(np.float32)
                  for b in range(B)]

    in_maps = []
    for c in range(NCORES):
        b, g = c // G, c % G
        m = {
            "xq": xqs[b], "xk": xks[b], "xv": xvs[b],
            "wq": wqs[g], "wk": wks[g], "wv": wvs[g],
            "tri2": tri2_np,
            "indh": indh_np,
        }
        if variant == "general":
            m["maskT"] = maskTs[b]
        in_maps.append(m)
    return variant, in_maps


def run(queries, keys, values, masks, Wq, Wk, Wv, Wo, bo, trace=False):
    from concourse import bass_utils
    import ml_dtypes
    bf16 = ml_dtypes.bfloat16

    queries = np.asarray(queries, np.float32)
    keys = np.asarray(keys, np.float32)
    values = np.asarray(values, np.float32)
    masks = np.asarray(masks, bool)
    Wq = np.asarray(Wq, np.float32)
    Wk = np.asarray(Wk, np.float32)
    Wv = np.asarray(Wv, np.float32)
    Wo = np.asarray(Wo, np.float32)
    bo = np.asarray(bo, np.float32)

    variant, in_maps = _host_prep(queries, keys, values, masks, Wq, Wk, Wv)
    wos = [np.ascontiguousarray(Wo[g * DH:(g + 1) * DH, :]).astype(bf16)
           for g in range(G)]
    for c in range(NCORES):
        in_maps[c]["wo"] = wos[c % G]

    nc = _get_program(variant)
    res = bass_utils.run_bass_kernel_spmd(
        nc, in_maps, list(range(NCORES)), trace=trace)

    out = np.empty((B, S, M), np.float32)
    for b in range(B):
        out[b] = res.results[G * b]["out"] + res.results[G * b + 1]["out"] + bo
    return out, res


def kernel(queries, keys, values, masks, Wq, Wk, Wv, Wo, bo):
    out, _ = run(queries, keys, values, masks, Wq, Wk, Wv, Wo, bo, trace=False)
    return out
